# revision 1
# baseline (speedup 1.0000x reference)
"""Trainium2 Bass kernel for BinaryMemoryTree logits.

logits[b,k,c] = sum_{d,e} q[b,k,d] * memory[b,c,d,e] * v[b,k,e]

Sharding: data-parallel over batch B=8 -> one batch element per NeuronCore.

Per-core algorithm (ARCH-1, fp32):
  - load q,v tiles naturally [128k, 128d]
  - PE-transpose q tiles -> q^T [d, k] (PSUM), ScalarE copy to SBUF
  - matmul: lhsT = q^T chunk (stationary), rhs = [M_0 | M_1] [d, 256] (fp32r)
      -> contextual [k, (c,e)] in PSUM
  - VectorE: P = contextual * v (v broadcast over c)
  - VectorE: segmented reduce over e -> logits [k, (t,c)]
  - DMA store
"""

import sys

sys.path.insert(0, "/opt/trn_rl_repo")

import numpy as np
from concourse import bacc, bass, bass_utils, masks, mybir, tile

B = 8
L = 32768
D = 128
C = 2
P = 128

F32 = mybir.dt.float32
F32R = mybir.dt.float32r
BF16 = mybir.dt.bfloat16
FP16 = mybir.dt.float16

import os as _os

MM_DTYPE = _os.environ.get("BMT_MM_DTYPE", "bf16")  # f32r | bf16 | f16
CTX_BUFS = int(_os.environ.get("BMT_CTX_BUFS", "3"))
VE_BF16 = _os.environ.get("BMT_VE_BF16", "0") == "1"  # bf16 2x-mode multiply
SCALE_RED_BLKS = int(_os.environ.get("BMT_SCALE_RED", "7"))  # blocks reduced on ScalarE
# (measured 359 us vs 160 us baseline: the ScalarE evacuation serializes the
#  MM->TT chain; keep the PSUM-direct fp32 multiply)

TILES = L // P          # 256 tiles of 128 queries
CHUNK_T = 4             # tiles per PSUM chunk (512 queries)
BLK_T = 16              # tiles per compute block (2048 queries)
NBLK = TILES // BLK_T   # 16 compute blocks
NCH = BLK_T // CHUNK_T  # 4 chunks per block
DMA_BLK = int(_os.environ.get("BMT_DMA_BLK", "1"))  # compute blocks per DMA block


def _kernel_body(tc, nc, qd, vd, md, od, stage="full"):
    ctxmgrs = []

    def pool(*args, **kw):
        p = tc.tile_pool(*args, **kw)
        ctxmgrs.append(p)
        return p.__enter__()

    constp = pool(name="const", bufs=1)
    iop = pool(name="io", bufs=2)
    qtps = pool(name="qt_ps", bufs=2, space="PSUM")
    ctxps = pool(name="ctx_ps", bufs=CTX_BUFS, space="PSUM")
    workp = pool(name="work", bufs=2)

    ident = constp.tile([P, P], F32)
    masks.make_identity(nc, ident[:])

    # M_cat [d, (c, e)]
    m_raw = constp.tile([P, C, D], F32)
    nc.sync.dma_start(m_raw[:], md.ap().transpose([1, 0, 2]))
    mm_dt = {"f32r": F32R, "bf16": BF16, "f16": FP16}[MM_DTYPE]
    m_sb = constp.tile([P, C, D], mm_dt)
    nc.scalar.copy(m_sb[:], m_raw[:])

    # p-major query mapping: k = p*(L//P) + g*(DMA_BLK*BLK_T) + b*BLK_T + t
    #  -> per-partition DMA runs are contiguous (DMA_BLK*BLK_T*512B loads, 2KB store)
    NG = NBLK // DMA_BLK
    q_view = qd.ap().rearrange(
        "(p g t) d -> g p t d", p=P, g=NG, t=DMA_BLK * BLK_T
    )
    v_view = vd.ap().rearrange(
        "(p g t) d -> g p t d", p=P, g=NG, t=DMA_BLK * BLK_T
    )
    o_view = od.ap().rearrange("(p j) c -> p j c", p=P)
    o_all = constp.tile([P, NBLK, BLK_T, C], F32)

    for blk in range(NBLK):
        g, b = divmod(blk, DMA_BLK)
        if b == 0:
            qg_sb = iop.tile([P, DMA_BLK * BLK_T, D], F32, tag="q")
            vg_sb = iop.tile([P, DMA_BLK * BLK_T, D], F32, tag="v")
            nc.sync.dma_start(qg_sb[:], q_view[g])
            nc.sync.dma_start(vg_sb[:], v_view[g])
        q_sb = qg_sb[:, b * BLK_T:(b + 1) * BLK_T, :]
        v_sb = vg_sb[:, b * BLK_T:(b + 1) * BLK_T, :]

        p_sb = workp.tile([P, BLK_T, C, D], BF16 if VE_BF16 else F32, tag="P")
        o_sb = o_all[:, blk]
        if VE_BF16 and stage == "full":
            vb_sb = workp.tile([P, BLK_T, D], BF16, tag="vb")
            nc.scalar.copy(vb_sb[:], v_sb[:])

        if stage == "dma":
            # touch inputs minimally so loads aren't dead
            nc.vector.tensor_reduce(
                out=o_sb[:, :, 0],
                in_=q_sb[:],
                axis=mybir.AxisListType.X,
                op=mybir.AluOpType.add,
            )
            nc.vector.tensor_reduce(
                out=o_sb[:, :, 1],
                in_=v_sb[:],
                axis=mybir.AxisListType.X,
                op=mybir.AluOpType.add,
            )
            continue

        for ch in range(NCH):
            qT = qtps.tile([P, CHUNK_T, P], F32, tag="qT")
            for t in range(CHUNK_T):
                tt = ch * CHUNK_T + t
                nc.tensor.transpose(qT[:, t, :], q_sb[:, tt, :], ident[:])
            qT_sb = workp.tile([P, CHUNK_T, P], mm_dt, tag="qTs")
            nc.scalar.copy(qT_sb[:], qT[:])

            if stage == "transpose":
                continue

            ctx = ctxps.tile([P, CHUNK_T, C, D], F32, tag="ctx")
            for t in range(CHUNK_T):
                nc.tensor.matmul(
                    ctx[:, t, :, :],
                    qT_sb[:, t, :],
                    m_sb[:],
                    start=True,
                    stop=True,
                )

            if stage == "matmul":
                # evacuate ctx cheaply so MMs aren't dead / serialized on PSUM
                nc.vector.tensor_reduce(
                    out=o_sb[:, ch * CHUNK_T:(ch + 1) * CHUNK_T, :],
                    in_=ctx[:],
                    axis=mybir.AxisListType.X,
                    op=mybir.AluOpType.add,
                )
                continue

            # P = contextual * v  (v broadcast over c)
            sl = slice(ch * CHUNK_T, (ch + 1) * CHUNK_T)
            if VE_BF16:
                # ScalarE evacuates ctx as bf16; DVE multiply runs SBUF
                # bf16 x bf16 in 2x_1P mode (per-c ops, unit-stride innermost)
                ctxb = workp.tile([P, CHUNK_T, C, D], BF16, tag="ctxb")
                nc.scalar.copy(ctxb[:], ctx[:])
                for c in range(C):
                    nc.vector.tensor_tensor(
                        out=p_sb[:, sl, c, :],
                        in0=ctxb[:, :, c, :],
                        in1=vb_sb[:, sl, :],
                        op=mybir.AluOpType.mult,
                    )
            else:
                v_b = v_sb[:, sl, :]
                v_bc = v_b.unsqueeze(2).broadcast_to([P, CHUNK_T, C, D])
                nc.vector.tensor_tensor(
                    out=p_sb[:, sl, :, :],
                    in0=ctx[:],
                    in1=v_bc,
                    op=mybir.AluOpType.mult,
                )

        if stage == "full":
            if blk < SCALE_RED_BLKS:
                # load-balance: ScalarE activation-accumulate per (t, c) group
                for t in range(BLK_T):
                    for c in range(C):
                        nc.scalar.activation(
                            out=p_sb[:, t, c, :],
                            in_=p_sb[:, t, c, :],
                            func=mybir.ActivationFunctionType.Copy,
                            accum_out=o_sb[:, t, c].unsqueeze(1),
                        )
            else:
                nc.vector.tensor_reduce(
                    out=o_sb[:],
                    in_=p_sb[:],
                    axis=mybir.AxisListType.X,
                    op=mybir.AluOpType.add,
                )

    # single contiguous store of all logits
    if stage != "transpose":
        nc.sync.dma_start(o_view, o_all[:].rearrange("p n t c -> p (n t) c"))

    for p in reversed(ctxmgrs):
        p.__exit__(None, None, None)


_NC_CACHE = {}


def _build(reps=1, stage="full"):
    key = ("nc", reps, stage, MM_DTYPE, DMA_BLK, CTX_BUFS, VE_BF16, SCALE_RED_BLKS)
    if key in _NC_CACHE:
        return _NC_CACHE[key]
    nc = bacc.Bacc("TRN2", target_bir_lowering=False, debug=False)
    qd = nc.dram_tensor("q", (L, D), F32, kind="ExternalInput")
    vd = nc.dram_tensor("v", (L, D), F32, kind="ExternalInput")
    md = nc.dram_tensor("m", (C, D, D), F32, kind="ExternalInput")
    od = nc.dram_tensor("o", (L, C), F32, kind="ExternalOutput")
    with tile.TileContext(nc) as tc:
        if reps == 1:
            _kernel_body(tc, nc, qd, vd, md, od, stage)
        else:
            with tc.For_i(0, reps, 1):
                _kernel_body(tc, nc, qd, vd, md, od, stage)
    nc.compile()
    _NC_CACHE[key] = nc
    return nc


def kernel(q, v, memory, _trace=False, _reps=1, _stage="full"):
    nc = _build(_reps, _stage)
    q = np.asarray(q, dtype=np.float32)
    v = np.asarray(v, dtype=np.float32)
    memory = np.asarray(memory, dtype=np.float32)
    in_maps = [
        {
            "q": np.ascontiguousarray(q[b]),
            "v": np.ascontiguousarray(v[b]),
            "m": np.ascontiguousarray(memory[b]),
        }
        for b in range(B)
    ]
    res = bass_utils.run_bass_kernel_spmd(
        nc, in_maps, core_ids=list(range(B)), trace=_trace
    )
    out = np.stack([res.results[b]["o"] for b in range(B)])
    if _trace:
        kernel.last_result = res
    return out



# revision 35
# speedup vs baseline: 2.4624x; 2.4624x over previous
"""Trainium2 Bass kernel for BinaryMemoryTree logits.

logits[b,k,c] = sum_{d,e} q[b,k,d] * memory[b,c,d,e] * v[b,k,e]

Sharding: data-parallel over batch B=8 -> one batch element per NeuronCore.

Per-core algorithm (fp32 in, bf16 matmul + products):
  - load q,v tiles naturally [128k, 128d]
  - PE-transpose q tiles -> q^T [d, k] (PSUM), ScalarE copy to SBUF (bf16)
  - matmul: lhsT = q^T tile (stationary), rhs = [M_0 | M_1] [d, 256] (bf16)
      -> contextual [k, (c,e)] in PSUM (fp32)
  - DVE: P = contextual * v -> SBUF bf16 (v broadcast over c)
  - reduce over e (default "pe" mode): 16 accumulating identity-matmuls on
    the PE fold e 128 -> 8 into PSUM (exact fp32 sums), then one cheap DVE
    tensor_reduce finishes 8 -> 1 per (t, c)
  - single DMA store of all logits
(legacy BMT_SPLIT=nD,nG,nA[,nE] selects DVE-tree/GPSIMD-tree/ACT-accum
 reduces per block instead; "pe" measured fastest: ~121us vs 135/163/195.)
"""

import sys

sys.path.insert(0, "/opt/trn_rl_repo")

import numpy as np
from concourse import bacc, bass, bass_utils, masks, mybir, tile

B = 8
L = 32768
D = 128
C = 2
P = 128

F32 = mybir.dt.float32
F32R = mybir.dt.float32r
BF16 = mybir.dt.bfloat16

import os as _os

TILES = L // P          # 256 tiles of 128 queries
CHUNK_T = 4             # tiles per PSUM chunk (512 queries)
BLK_T = 16              # tiles per compute block (2048 queries)
NBLK = TILES // BLK_T   # 16 compute blocks
NCH = BLK_T // CHUNK_T  # 4 chunks per block
DMA_BLK = int(_os.environ.get("BMT_DMA_BLK", "1"))  # compute blocks per DMA block

# Reduce-engine split over the 16 blocks: nD on DVE-tree, nG on GPSIMD-tree,
# nA on ScalarE-accum.  DVE also does every block's multiply, so it gets the
# fewest reduces; ACT also does the qT evacuations.
_SPLIT = _os.environ.get("BMT_SPLIT", "pe")
if _SPLIT == "pe":
    N_DVE = N_GPS = N_ACT = N_E2E = 0
else:
    _split_parts = [int(x) for x in _SPLIT.split(",")]
    if len(_split_parts) == 3:
        _split_parts.append(0)
    N_DVE, N_GPS, N_ACT, N_E2E = _split_parts
    assert N_DVE + N_GPS + N_ACT + N_E2E == NBLK
P_BUFS = int(_os.environ.get("BMT_P_BUFS", "4"))
TREE_BUFS = int(_os.environ.get("BMT_TREE_BUFS", "3"))
# PSUM pool shapes for the pe-reduce config (must total <= 8 banks):
#   qT: QT_BUFS banks, ctx: 2*CTX_BUFS banks, red: RED_BUFS banks
QT_BUFS = int(_os.environ.get("BMT_QT_BUFS", "2"))
CTX_BUFS = int(_os.environ.get("BMT_CTX_BUFS", "2"))
RED_BUFS = int(_os.environ.get("BMT_RED_BUFS", "2"))
RED_SLICE = int(_os.environ.get("BMT_RED_SLICE", "8"))  # e per reduce-MM
V_RING = _os.environ.get("BMT_V_RING", "sync")  # sync | scalar HWDGE ring for v


ORDER = _os.environ.get("BMT_ORDER", "tail_dve")


def _assign_blocks():
    if _SPLIT == "pe":
        return ["pe"] * NBLK
    if ORDER == "even":
        # interleave the three classes as evenly as possible
        slots = []
        for name, n in (("gps", N_GPS), ("dve", N_DVE), ("act", N_ACT)):
            for i in range(n):
                slots.append(((i + 0.5) / n, name))
        slots.sort()
        return [name for _, name in slots]
    # tail_dve: GPSIMD trees lag the multiply by up to P_BUFS-1 blocks, so
    # front-load them; finish with ACT accums (inline) and DVE trees (fast,
    # same-engine-inline) to avoid a serialized GPSIMD tail before the store.
    return (
        ["gpse2e"] * N_E2E + ["gps"] * N_GPS + ["act"] * N_ACT + ["dve"] * N_DVE
    )


ENGINE_OF = _assign_blocks()


def _kernel_body(tc, nc, qd, vd, md, od, stage="full"):
    ctxmgrs = []

    def pool(*args, **kw):
        p = tc.tile_pool(*args, **kw)
        ctxmgrs.append(p)
        return p.__enter__()

    use_pe_red = stage in ("full", "mult") and any(
        e == "pe" for e in ENGINE_OF
    )
    constp = pool(name="const", bufs=1)
    iop = pool(name="io", bufs=2)
    qtps = pool(name="qt_ps", bufs=QT_BUFS, space="PSUM")
    ctxps = pool(
        name="ctx_ps", bufs=CTX_BUFS if use_pe_red else 3, space="PSUM"
    )
    if use_pe_red:
        redps = pool(name="red_ps", bufs=RED_BUFS, space="PSUM")
    workp = pool(name="work", bufs=2)
    treep = pool(name="tree", bufs=2)

    ident = constp.tile([P, P], F32)
    masks.make_identity(nc, ident[:])
    if use_pe_red:
        ident_bf = constp.tile([P, P], BF16)
        nc.scalar.copy(ident_bf[:], ident[:])

    # M_cat [d, (c, e)]
    m_raw = constp.tile([P, C, D], F32)
    nc.sync.dma_start(m_raw[:], md.ap().transpose([1, 0, 2]))
    m_sb = constp.tile([P, C, D], BF16)
    nc.scalar.copy(m_sb[:], m_raw[:])

    # p-major query mapping: k = p*(L//P) + g*(DMA_BLK*BLK_T) + b*BLK_T + t
    #  -> per-partition DMA runs are contiguous
    NG = NBLK // DMA_BLK
    q_view = qd.ap().rearrange(
        "(p g t) d -> g p t d", p=P, g=NG, t=DMA_BLK * BLK_T
    )
    v_view = vd.ap().rearrange(
        "(p g t) d -> g p t d", p=P, g=NG, t=DMA_BLK * BLK_T
    )
    o_view = od.ap().rearrange("(p j) c -> p j c", p=P)
    o_all = constp.tile([P, NBLK, BLK_T, C], F32)
    V_ENG = nc.scalar if V_RING == "scalar" else nc.sync

    for blk in range(NBLK):
        g, b = divmod(blk, DMA_BLK)
        if b == 0:
            qg_sb = iop.tile([P, DMA_BLK * BLK_T, D], F32, tag="q")
            vg_sb = iop.tile([P, DMA_BLK * BLK_T, D], F32, tag="v")
            nc.sync.dma_start(qg_sb[:], q_view[g])
            V_ENG.dma_start(vg_sb[:], v_view[g])
        q_sb = qg_sb[:, b * BLK_T:(b + 1) * BLK_T, :]
        v_sb = vg_sb[:, b * BLK_T:(b + 1) * BLK_T, :]

        o_sb = o_all[:, blk]
        if stage == "dma":
            # touch inputs minimally so loads aren't dead
            nc.vector.tensor_reduce(
                out=o_sb[:, :, 0],
                in_=q_sb[:],
                axis=mybir.AxisListType.X,
                op=mybir.AluOpType.add,
            )
            nc.vector.tensor_reduce(
                out=o_sb[:, :, 1],
                in_=v_sb[:],
                axis=mybir.AxisListType.X,
                op=mybir.AluOpType.add,
            )
            continue

        red_eng = ENGINE_OF[blk]
        # deep-buffered: the reduce (esp. GPSIMD tree) lags the multiply by
        # up to P_BUFS-1 blocks without stalling it
        p_sb = workp.tile([P, BLK_T, C, D], BF16, tag="P", bufs=P_BUFS)

        for ch in range(NCH):
            qT = qtps.tile([P, CHUNK_T, P], F32, tag="qT")
            for t in range(CHUNK_T):
                tt = ch * CHUNK_T + t
                nc.tensor.transpose(qT[:, t, :], q_sb[:, tt, :], ident[:])
            qT_sb = workp.tile([P, CHUNK_T, P], BF16, tag="qTs")
            nc.scalar.copy(qT_sb[:], qT[:])

            if stage == "transpose":
                continue

            ctx = ctxps.tile([P, CHUNK_T, C, D], F32, tag="ctx")
            for t in range(CHUNK_T):
                nc.tensor.matmul(
                    ctx[:, t, :, :],
                    qT_sb[:, t, :],
                    m_sb[:],
                    start=True,
                    stop=True,
                )

            if stage == "matmul":
                # evacuate ctx cheaply so MMs aren't dead / serialized on PSUM
                nc.vector.tensor_reduce(
                    out=o_sb[:, ch * CHUNK_T:(ch + 1) * CHUNK_T, :],
                    in_=ctx[:],
                    axis=mybir.AxisListType.X,
                    op=mybir.AluOpType.add,
                )
                continue

            # interleave previous chunk's ACT accumulates between this
            # chunk's ACT work, so ACT's FIFO never has a long burst that
            # stalls the next qT evacuation (which gates the PE)
            if stage == "full" and red_eng == "act" and ch > 0:
                pv = ch - 1
                for t in range(pv * CHUNK_T, (pv + 1) * CHUNK_T):
                    for c in range(C):
                        nc.scalar.activation(
                            out=p_sb[:, t, c, :],
                            in_=p_sb[:, t, c, :],
                            func=mybir.ActivationFunctionType.Copy,
                            accum_out=o_sb[:, t, c].unsqueeze(1),
                        )

            sl = slice(ch * CHUNK_T, (ch + 1) * CHUNK_T)
            v_b = v_sb[:, sl, :]
            v_bc = v_b.unsqueeze(2).broadcast_to([P, CHUNK_T, C, D])
            if stage == "full" and red_eng == "gpse2e":
                # decouple from DVE entirely: ACT evacuates ctx -> SBUF bf16,
                # GPSIMD does the multiply (and later the tree)
                if ch == 0:
                    ctx_sb = workp.tile(
                        [P, BLK_T, C, D], BF16, tag="ctxsb", bufs=2
                    )
                nc.scalar.copy(ctx_sb[:, sl, :, :], ctx[:])
                nc.gpsimd.tensor_tensor(
                    out=p_sb[:, sl, :, :],
                    in0=ctx_sb[:, sl, :, :],
                    in1=v_bc,
                    op=mybir.AluOpType.mult,
                )
            else:
                # P = contextual * v  (v broadcast over c) -> bf16 (DVE)
                nc.vector.tensor_tensor(
                    out=p_sb[:, sl, :, :],
                    in0=ctx[:],
                    in1=v_bc,
                    op=mybir.AluOpType.mult,
                )

        if stage == "mult":
            # keep o_all written, negligible cost
            nc.vector.tensor_reduce(
                out=o_sb[:, :, :],
                in_=p_sb[:, :, :, 0:2],
                axis=mybir.AxisListType.X,
                op=mybir.AluOpType.add,
            )
            continue
        if stage != "full":
            continue

        if red_eng == "pe":
            # PE-assisted reduce: 8 accumulating identity-matmuls fold the
            # e axis 128 -> 16 into PSUM (exact fp32 sums), then one cheap
            # DVE tensor_reduce finishes 16 -> 1.
            nsl = D // RED_SLICE
            red = redps.tile([P, BLK_T, C, RED_SLICE], F32, tag="red")
            for j in range(nsl):
                nc.tensor.matmul(
                    red[:],
                    ident_bf[:],
                    p_sb[:, :, :, RED_SLICE * j:RED_SLICE * (j + 1)],
                    start=(j == 0),
                    stop=(j == nsl - 1),
                )
            nc.vector.tensor_reduce(
                out=o_sb[:],
                in_=red[:],
                axis=mybir.AxisListType.X,
                op=mybir.AluOpType.add,
            )
        elif red_eng == "act":
            # ScalarE: last chunk's segmented accumulates
            for t in range((NCH - 1) * CHUNK_T, NCH * CHUNK_T):
                for c in range(C):
                    nc.scalar.activation(
                        out=p_sb[:, t, c, :],
                        in_=p_sb[:, t, c, :],
                        func=mybir.ActivationFunctionType.Copy,
                        accum_out=o_sb[:, t, c].unsqueeze(1),
                    )
        else:
            # binary tree of bf16 adds (DVE 2x mode, or GPSIMD as 3rd engine)
            eng = nc.vector if red_eng == "dve" else nc.gpsimd
            assert red_eng in ("dve", "gps", "gpse2e")
            s = treep.tile([P, BLK_T, C, D], BF16, tag="tr", bufs=TREE_BUFS)
            a = p_sb
            eng.tensor_tensor(
                out=s[:, :, :, 0:64], in0=a[:, :, :, 0:64], in1=a[:, :, :, 64:128],
                op=mybir.AluOpType.add)
            eng.tensor_tensor(
                out=s[:, :, :, 64:96], in0=s[:, :, :, 0:32], in1=s[:, :, :, 32:64],
                op=mybir.AluOpType.add)
            eng.tensor_tensor(
                out=s[:, :, :, 96:112], in0=s[:, :, :, 64:80], in1=s[:, :, :, 80:96],
                op=mybir.AluOpType.add)
            eng.tensor_tensor(
                out=s[:, :, :, 112:120], in0=s[:, :, :, 96:104], in1=s[:, :, :, 104:112],
                op=mybir.AluOpType.add)
            eng.tensor_tensor(
                out=s[:, :, :, 120:124], in0=s[:, :, :, 112:116], in1=s[:, :, :, 116:120],
                op=mybir.AluOpType.add)
            eng.tensor_tensor(
                out=s[:, :, :, 124:126], in0=s[:, :, :, 120:122], in1=s[:, :, :, 122:124],
                op=mybir.AluOpType.add)
            eng.tensor_tensor(
                out=o_sb[:, :, :].unsqueeze(3), in0=s[:, :, :, 124:125],
                in1=s[:, :, :, 125:126],
                op=mybir.AluOpType.add)

    # single contiguous store of all logits
    if stage != "transpose":
        nc.sync.dma_start(o_view, o_all[:].rearrange("p n t c -> p (n t) c"))

    for p in reversed(ctxmgrs):
        p.__exit__(None, None, None)


_NC_CACHE = {}


def _build(reps=1, stage="full"):
    key = ("nc", reps, stage, DMA_BLK, _SPLIT, P_BUFS, TREE_BUFS, QT_BUFS, CTX_BUFS, RED_BUFS, RED_SLICE, V_RING)
    if key in _NC_CACHE:
        return _NC_CACHE[key]
    nc = bacc.Bacc("TRN2", target_bir_lowering=False, debug=False)
    qd = nc.dram_tensor("q", (L, D), F32, kind="ExternalInput")
    vd = nc.dram_tensor("v", (L, D), F32, kind="ExternalInput")
    md = nc.dram_tensor("m", (C, D, D), F32, kind="ExternalInput")
    od = nc.dram_tensor("o", (L, C), F32, kind="ExternalOutput")
    with tile.TileContext(nc) as tc:
        if reps == 1:
            _kernel_body(tc, nc, qd, vd, md, od, stage)
        else:
            with tc.For_i(0, reps, 1):
                _kernel_body(tc, nc, qd, vd, md, od, stage)
    nc.compile()
    _NC_CACHE[key] = nc
    return nc


def kernel(q, v, memory, _trace=False, _reps=1, _stage="full"):
    nc = _build(_reps, _stage)
    q = np.asarray(q, dtype=np.float32)
    v = np.asarray(v, dtype=np.float32)
    memory = np.asarray(memory, dtype=np.float32)
    in_maps = [
        {
            "q": np.ascontiguousarray(q[b]),
            "v": np.ascontiguousarray(v[b]),
            "m": np.ascontiguousarray(memory[b]),
        }
        for b in range(B)
    ]
    res = bass_utils.run_bass_kernel_spmd(
        nc, in_maps, core_ids=list(range(B)), trace=_trace
    )
    out = np.stack([res.results[b]["o"] for b in range(B)])
    if _trace:
        kernel.last_result = res
    return out
